# revision 16
# baseline (speedup 1.0000x reference)
"""GNN message-passing (NodeModel) kernel for 8 Trainium2 NeuronCores.

Strategy:
  * Host: sort edges by receiver (argsort of the int32 index array), bucket
    them into 128-node tiles, and order each core's tiles by descending edge
    count so all 8 cores share one program whose per-position capacity
    Cs[t] = max over cores of the t-th largest tile count (minimal padding).
    Data is laid out partition-major; node tiles are sharded contiguously
    across the 8 cores and un-permuted on the host afterwards.
  * Device (one NEFF shared by all cores): for each node tile, build all
    Cs[t] one-hot blocks with a single wide is_equal tensor_tensor on
    VectorE (receiver values are duplicated in pairs so the broadcast
    operand keeps a step-1 innermost dim and the DVE 2x packed mode
    engages), scatter-add edge features via TensorE matmuls into PSUM,
    producing agg^T [feat, node]. The node MLP is fused right behind it:
    h^T blocks = W1a^T@agg^T + W1b^T@nodes^T (PSUM), ReLU on ScalarE,
    out = h @ W2, DMA out. The per-tile stages are software-pipelined
    (scatter(t) | layer1(t-1) | layer2(t-2)) so the PE never waits on the
    ScalarE evacuations. Biases are folded in via K=1 ones-row matmuls only
    when they are nonzero (the common all-zero case skips them).
  * No collectives needed: every core owns a disjoint node range.

Compute dtype is bf16 with fp32 PSUM accumulation (end-to-end rel err ~4e-3).
"""

import os
import sys

import numpy as np
import ml_dtypes

if "/opt/trn_rl_repo" not in sys.path:
    sys.path.insert(0, "/opt/trn_rl_repo")

P = 128
N_NODES = 50000
N_EDGES = 800000
F = 128          # edge/node feature dim
HID = 512
HB = HID // P    # 4 hidden blocks
FO = 128         # output feature dim
NCORES = 8
TILES_TOTAL = (N_NODES + P - 1) // P                      # 391
TILES_PAD = ((TILES_TOTAL + NCORES - 1) // NCORES) * NCORES  # 392
T = TILES_PAD // NCORES                                   # 49 node tiles per core
NNC = T * P                                               # 6272 nodes per core
NPAD = TILES_PAD * P                                      # 50176
PREFETCH = 6     # edge-tile DMAs issued ahead of consumption

_NC_CACHE: dict = {}
LAST_RESULTS = None  # stash of BassKernelResults for test harness introspection


def _build(Cs: tuple, with_b1: bool, with_b2: bool):
    """Build + compile the per-core Bass program.

    Cs[t]: number of 128-edge groups for tile position t. with_b1/with_b2:
    emit bias adds (skipped when the bias vectors are exactly zero).
    """
    from contextlib import ExitStack

    import concourse.mybir as mybir
    import concourse.tile as tile
    from concourse import bacc

    bf = mybir.dt.bfloat16
    f32 = mybir.dt.float32

    EC = int(sum(Cs))                 # total edge groups per core
    eoff = np.concatenate([[0], np.cumsum(Cs)]).astype(int)
    Cmax = int(max(Cs))

    nc = bacc.Bacc(
        "TRN2",
        target_bir_lowering=False,
        debug=False,
        enable_asserts=False,
        num_devices=NCORES,
    )

    edges_d = nc.dram_tensor("edges", [P, EC * F], bf, kind="ExternalInput")
    # receiver-relative values, duplicated in adjacent pairs (see below)
    rrel_d = nc.dram_tensor("rrel", [P, EC * 2], bf, kind="ExternalInput")
    # iota constant: value n at column c*P + n (n in [0,P)), any c
    iotac_d = nc.dram_tensor("iotac", [P, Cmax * P], bf, kind="ExternalInput")
    nodesT_d = nc.dram_tensor("nodesT", [P, NNC], bf, kind="ExternalInput")
    w1a_d = nc.dram_tensor("w1a", [P, HID], bf, kind="ExternalInput")
    w1b_d = nc.dram_tensor("w1b", [P, HID], bf, kind="ExternalInput")
    w2_d = nc.dram_tensor("w2", [P, HB, FO], bf, kind="ExternalInput")
    if with_b1:
        b1r_d = nc.dram_tensor("b1r", [1, HID], bf, kind="ExternalInput")
    if with_b2:
        b2_d = nc.dram_tensor("b2", [1, FO], bf, kind="ExternalInput")
    out_d = nc.dram_tensor("out", [NNC, FO], f32, kind="ExternalOutput")

    with tile.TileContext(nc) as tc, ExitStack() as ctx:
        const = ctx.enter_context(tc.tile_pool(name="const", bufs=1))
        epool = ctx.enter_context(tc.tile_pool(name="edges", bufs=PREFETCH + 2))
        ohpool = ctx.enter_context(tc.tile_pool(name="oh", bufs=3))
        spool = ctx.enter_context(tc.tile_pool(name="sb", bufs=3))
        pagg = ctx.enter_context(tc.tile_pool(name="pagg", bufs=2, space="PSUM"))
        ph = ctx.enter_context(tc.tile_pool(name="ph", bufs=2, space="PSUM"))
        pout = ctx.enter_context(tc.tile_pool(name="pout", bufs=2, space="PSUM"))

        # Resident loads. Two HWDGE rings are used: edge/receiver data on the
        # sync ring, constants + outputs on the scalar ring, so the first
        # edge tiles are not queued behind the big constant transfers.
        # rrel is split head/tail so the first one-hot ops only wait on the
        # small head transfer.
        HEAD = min(PREFETCH, T)
        hc = int(eoff[HEAD]) * 2
        rrel_h = const.tile([P, hc], bf)
        nc.sync.dma_start(rrel_h[:], rrel_d[:, :hc])
        rrel_tl = const.tile([P, EC * 2 - hc], bf)
        iota_t = const.tile([P, Cmax * P], bf)
        nc.scalar.dma_start(iota_t[:], iotac_d[:])
        w1a_t = const.tile([P, HID], bf)
        nc.scalar.dma_start(w1a_t[:], w1a_d[:])
        w1b_t = const.tile([P, HID], bf)
        nc.scalar.dma_start(w1b_t[:], w1b_d[:])
        w2_t = const.tile([P, HB, FO], bf)
        nc.scalar.dma_start(w2_t[:], w2_d[:])
        if with_b1:
            b1r_t = const.tile([1, HID], bf)
            nc.scalar.dma_start(b1r_t[:], b1r_d[:])
        if with_b2:
            b2_t = const.tile([1, FO], bf)
            nc.scalar.dma_start(b2_t[:], b2_d[:])

        def rrel_slice(t):
            a, b = eoff[t] * 2, (eoff[t] + Cs[t]) * 2
            if b <= hc:
                return rrel_h[:, a:b]
            return rrel_tl[:, a - hc : b - hc]
        if with_b1 or with_b2:
            ones_t = const.tile([1, P], bf)
            nc.vector.memset(ones_t[:], 1.0)

        et_tiles = {}

        def load_edges(t):
            c = Cs[t]
            et = epool.tile([P, c * F], bf, tag="et", name=f"et{t}")
            nc.sync.dma_start(et[:], edges_d[:, eoff[t] * F : (eoff[t] + c) * F])
            et_tiles[t] = et

        for t in range(min(PREFETCH, T)):
            load_edges(t)

        nc.sync.dma_start(rrel_tl[:], rrel_d[:, hc:])
        nodesT_t = const.tile([P, NNC], bf)
        nc.scalar.dma_start(nodesT_t[:], nodesT_d[:])

        # Software pipeline across node tiles so the PE never waits on the
        # ScalarE evacuations: iteration t runs scatter(t), layer1(t-1),
        # layer2(t-2) back-to-back on the PE.
        agg_sb = [None] * T
        h_sb = [None] * T

        def stage_scatter(t):
            c = Cs[t]
            et = et_tiles.pop(t)

            # one-hot blocks for the whole node tile in one DVE op; operands
            # viewed as [P, c, 64, 2] so every stream has a step-1 innermost
            # pair (rrel values are duplicated on the host).
            oh = ohpool.tile([P, c, P], bf, tag="oh", name=f"oh{t}")
            rr_bc = (
                rrel_slice(t)
                .rearrange("p (c r) -> p c r", r=2)[:, :, None, :]
                .to_broadcast([P, c, P // 2, 2])
            )
            nc.vector.tensor_tensor(
                out=oh[:].rearrange("p c (q r) -> p c q r", r=2),
                in0=iota_t[:, : c * P].rearrange("p (c q r) -> p c q r", r=2, q=P // 2),
                in1=rr_bc,
                op=mybir.AluOpType.is_equal,
            )

            # scatter-add into agg^T [feat, node] via one-hot matmuls
            agg_ps = pagg.tile([P, P], f32, tag="agg", name=f"agg_ps{t}")
            for j in range(c):
                nc.tensor.matmul(
                    agg_ps[:],
                    lhsT=et[:, j * F : (j + 1) * F],
                    rhs=oh[:, j, :],
                    start=(j == 0),
                    stop=(j == c - 1),
                )
            agg_sb[t] = spool.tile([P, P], bf, tag="agg_sb", name=f"agg_sb{t}")
            nc.scalar.activation(
                agg_sb[t][:], agg_ps[:], mybir.ActivationFunctionType.Copy
            )

        def stage_l1(t):
            h_ps = ph.tile([P, HB, P], f32, tag="h_ps", name=f"h_ps{t}")
            for hb in range(HB):
                nc.tensor.matmul(
                    h_ps[:, hb, :],
                    lhsT=w1a_t[:, hb * P : (hb + 1) * P],
                    rhs=agg_sb[t][:],
                    start=True,
                    stop=False,
                )
                nc.tensor.matmul(
                    h_ps[:, hb, :],
                    lhsT=w1b_t[:, hb * P : (hb + 1) * P],
                    rhs=nodesT_t[:, t * P : (t + 1) * P],
                    start=False,
                    stop=(not with_b1),
                )
                if with_b1:
                    nc.tensor.matmul(
                        h_ps[:, hb, :],
                        lhsT=b1r_t[:, hb * P : (hb + 1) * P],
                        rhs=ones_t[:],
                        start=False,
                        stop=True,
                    )
            h_sb[t] = spool.tile([P, HB, P], bf, tag="h_sb", name=f"h_sb{t}")
            nc.scalar.activation(
                h_sb[t][:], h_ps[:], mybir.ActivationFunctionType.Relu
            )

        def stage_l2(t):
            o_ps = pout.tile([P, FO], f32, tag="o_ps", name=f"o_ps{t}")
            for hb in range(HB):
                nc.tensor.matmul(
                    o_ps[:],
                    lhsT=h_sb[t][:, hb, :],
                    rhs=w2_t[:, hb, :],
                    start=(hb == 0),
                    stop=(hb == HB - 1 and not with_b2),
                )
            if with_b2:
                nc.tensor.matmul(
                    o_ps[:], lhsT=ones_t[:], rhs=b2_t[:], start=False, stop=True
                )
            o_sb = spool.tile([P, FO], f32, tag="o_sb", name=f"o_sb{t}")
            nc.vector.tensor_copy(o_sb[:], o_ps[:])
            nc.scalar.dma_start(out_d[t * P : (t + 1) * P, :], o_sb[:])

        for t in range(T + 2):
            if t + PREFETCH < T:
                load_edges(t + PREFETCH)
            if t < T:
                stage_scatter(t)
            if 1 <= t <= T:
                stage_l1(t - 1)
            if 2 <= t <= T + 1:
                stage_l2(t - 2)

    nc.compile()
    return nc


def _get_nc(Cs: tuple, with_b1: bool, with_b2: bool):
    key = (Cs, with_b1, with_b2)
    if key not in _NC_CACHE:
        _NC_CACHE[key] = _build(*key)
    return _NC_CACHE[key]


def kernel(nodes, edge_attr, senders, receivers, W1, b1, W2, b2):
    global LAST_RESULTS
    from concourse.bass_utils import run_bass_kernel_spmd

    bfnp = ml_dtypes.bfloat16
    nodes = np.asarray(nodes, dtype=np.float32)
    edge_attr = np.asarray(edge_attr, dtype=np.float32)
    receivers = np.asarray(receivers, dtype=np.int32)
    W1 = np.asarray(W1, dtype=np.float32)
    b1 = np.asarray(b1, dtype=np.float32)
    W2 = np.asarray(W2, dtype=np.float32)
    b2 = np.asarray(b2, dtype=np.float32)
    with_b1 = bool(np.any(b1 != 0))
    with_b2 = bool(np.any(b2 != 0))

    # ---- host-side prep: sort edges by receiver, order tiles, pad ----
    perm = np.argsort(receivers, kind="stable")
    recv_s = receivers[perm]
    bounds = np.searchsorted(recv_s, np.arange(TILES_PAD + 1) * P)
    counts = np.diff(bounds)                       # [TILES_PAD]
    counts_core = counts.reshape(NCORES, T)
    order = np.argsort(-counts_core, axis=1, kind="stable")  # [NCORES, T]
    sorted_counts = np.take_along_axis(counts_core, order, axis=1)
    Cs = np.maximum(np.ceil(sorted_counts.max(axis=0) / P).astype(int), 1)
    Cs_t = tuple(int(x) for x in Cs)
    Cmax = int(Cs.max())
    eoff = np.concatenate([[0], np.cumsum(Cs)]).astype(int)
    EC = int(eoff[-1])

    # per-tile padded edge data/receiver arrays at width Cmax (then sliced)
    EPT = Cmax * P
    slot = np.arange(EPT)[None, :]
    mask = slot < counts[:, None]  # [TILES_PAD, EPT]
    pidx = np.zeros((TILES_PAD, EPT), np.int64)
    pidx[mask] = perm
    rrel = np.full((TILES_PAD, EPT), -1.0, np.float32)
    rrel[mask] = (recv_s % P).astype(np.float32)

    eb = edge_attr.astype(bfnp)
    g = eb[pidx.reshape(-1)].reshape(TILES_PAD, Cmax, P, F)
    g[~mask.reshape(TILES_PAD, Cmax, P)] = 0
    g = g.reshape(NCORES, T, Cmax, P, F)
    rrel = rrel.reshape(NCORES, T, Cmax, P)

    edges_dev = np.empty((NCORES, P, EC * F), bfnp)
    rr2 = np.empty((NCORES, P, EC * 2), bfnp)
    for c in range(NCORES):
        for t in range(T):
            o = order[c, t]
            w = Cs[t]
            blk = g[c, o, :w]                      # [w, P, F]
            edges_dev[c, :, eoff[t] * F : (eoff[t] + w) * F] = (
                blk.transpose(1, 0, 2).reshape(P, w * F)
            )
            rb = rrel[c, o, :w].astype(bfnp)       # [w, P]
            rr2[c, :, eoff[t] * 2 : (eoff[t] + w) * 2] = np.repeat(
                rb.T, 2, axis=1
            )

    iotac = np.tile(np.arange(P, dtype=np.float32), Cmax)[None].repeat(P, 0)
    iotac = iotac.astype(bfnp)                     # [P, Cmax*P]

    nodes_pad = np.zeros((NPAD, F), np.float32)
    nodes_pad[:N_NODES] = nodes
    nodes_core = nodes_pad.reshape(NCORES, T, P, F)
    nodesT_dev = np.empty((NCORES, P, NNC), bfnp)
    for c in range(NCORES):
        sel = nodes_core[c, order[c]].reshape(NNC, F)  # position-ordered
        nodesT_dev[c] = sel.T.astype(bfnp)

    w1a = np.ascontiguousarray(W1[:F]).astype(bfnp)
    w1b = np.ascontiguousarray(W1[F:]).astype(bfnp)
    w2 = np.ascontiguousarray(W2.reshape(HB, P, FO).transpose(1, 0, 2)).astype(bfnp)

    nc = _get_nc(Cs_t, with_b1, with_b2)

    in_maps = []
    for c in range(NCORES):
        m = {
            "edges": edges_dev[c],
            "rrel": rr2[c],
            "iotac": iotac,
            "nodesT": nodesT_dev[c],
            "w1a": w1a,
            "w1b": w1b,
            "w2": w2,
        }
        if with_b1:
            m["b1r"] = b1.reshape(1, HID).astype(bfnp)
        if with_b2:
            m["b2"] = b2.reshape(1, FO).astype(bfnp)
        in_maps.append(m)

    trace = bool(int(os.environ.get("GNN_TRACE", "0")))
    res = run_bass_kernel_spmd(
        nc,
        in_maps,
        core_ids=list(range(NCORES)),
        trace=trace,
    )
    LAST_RESULTS = res

    # un-permute tile positions back to original node order
    out = np.empty((NPAD, FO), np.float32)
    for c in range(NCORES):
        r = res.results[c]["out"].reshape(T, P, FO)
        out.reshape(NCORES, T, P, FO)[c, order[c]] = r
    return np.ascontiguousarray(out[:N_NODES])


# revision 17
# speedup vs baseline: 1.0561x; 1.0561x over previous
"""GNN message-passing (NodeModel) kernel for 8 Trainium2 NeuronCores.

Strategy:
  * Host: sort edges by receiver (argsort of the int32 index array), bucket
    them into 128-node tiles, and order each core's tiles by descending edge
    count so all 8 cores share one program whose per-position capacity
    Cs[t] = max over cores of the t-th largest tile count (minimal padding).
    Data is laid out partition-major; node tiles are sharded contiguously
    across the 8 cores and un-permuted on the host afterwards.
  * Device (one NEFF shared by all cores): for each node tile, build all
    Cs[t] one-hot blocks with a single wide is_equal tensor_tensor on
    VectorE (receiver values are duplicated in pairs so the broadcast
    operand keeps a step-1 innermost dim and the DVE 2x packed mode
    engages), scatter-add edge features via TensorE matmuls into PSUM,
    producing agg^T [feat, node]. The node MLP is fused right behind it:
    h^T blocks = W1a^T@agg^T + W1b^T@nodes^T (PSUM), ReLU on ScalarE,
    out = h @ W2, DMA out. The per-tile stages are software-pipelined
    (scatter(t) | layer1(t-1) | layer2(t-2)) so the PE never waits on the
    ScalarE evacuations. Biases are folded in via K=1 ones-row matmuls only
    when they are nonzero (the common all-zero case skips them).
  * No collectives needed: every core owns a disjoint node range.

Compute dtype is bf16 with fp32 PSUM accumulation (end-to-end rel err ~4e-3).
"""

import os
import sys

import numpy as np
import ml_dtypes

if "/opt/trn_rl_repo" not in sys.path:
    sys.path.insert(0, "/opt/trn_rl_repo")

P = 128
N_NODES = 50000
N_EDGES = 800000
F = 128          # edge/node feature dim
HID = 512
HB = HID // P    # 4 hidden blocks
FO = 128         # output feature dim
NCORES = 8
TILES_TOTAL = (N_NODES + P - 1) // P                      # 391
TILES_PAD = ((TILES_TOTAL + NCORES - 1) // NCORES) * NCORES  # 392
T = TILES_PAD // NCORES                                   # 49 node tiles per core
NNC = T * P                                               # 6272 nodes per core
NPAD = TILES_PAD * P                                      # 50176
PREFETCH = 6     # edge-tile DMAs issued ahead of consumption

_NC_CACHE: dict = {}
LAST_RESULTS = None  # stash of BassKernelResults for test harness introspection


def _build(Cs: tuple, with_b1: bool, with_b2: bool):
    """Build + compile the per-core Bass program.

    Cs[t]: number of 128-edge groups for tile position t. with_b1/with_b2:
    emit bias adds (skipped when the bias vectors are exactly zero).
    """
    from contextlib import ExitStack

    import concourse.mybir as mybir
    import concourse.tile as tile
    from concourse import bacc

    bf = mybir.dt.bfloat16
    f32 = mybir.dt.float32

    EC = int(sum(Cs))                 # total edge groups per core
    eoff = np.concatenate([[0], np.cumsum(Cs)]).astype(int)
    Cmax = int(max(Cs))

    nc = bacc.Bacc(
        "TRN2",
        target_bir_lowering=False,
        debug=False,
        enable_asserts=False,
        num_devices=NCORES,
    )

    edges_d = nc.dram_tensor("edges", [P, EC * F], bf, kind="ExternalInput")
    # receiver-relative values, duplicated in adjacent pairs (see below)
    rrel_d = nc.dram_tensor("rrel", [P, EC * 2], bf, kind="ExternalInput")
    # iota constant: value n at column c*P + n (n in [0,P)), any c
    iotac_d = nc.dram_tensor("iotac", [P, Cmax * P], bf, kind="ExternalInput")
    nodesT_d = nc.dram_tensor("nodesT", [P, NNC], bf, kind="ExternalInput")
    w1a_d = nc.dram_tensor("w1a", [P, HID], bf, kind="ExternalInput")
    w1b_d = nc.dram_tensor("w1b", [P, HID], bf, kind="ExternalInput")
    w2_d = nc.dram_tensor("w2", [P, HB, FO], bf, kind="ExternalInput")
    if with_b1:
        b1r_d = nc.dram_tensor("b1r", [1, HID], bf, kind="ExternalInput")
    if with_b2:
        b2_d = nc.dram_tensor("b2", [1, FO], bf, kind="ExternalInput")
    out_d = nc.dram_tensor("out", [NNC, FO], f32, kind="ExternalOutput")

    with tile.TileContext(nc) as tc, ExitStack() as ctx:
        const = ctx.enter_context(tc.tile_pool(name="const", bufs=1))
        epool = ctx.enter_context(tc.tile_pool(name="edges", bufs=PREFETCH + 2))
        ohpool = ctx.enter_context(tc.tile_pool(name="oh", bufs=3))
        spool = ctx.enter_context(tc.tile_pool(name="sb", bufs=3))
        pagg = ctx.enter_context(tc.tile_pool(name="pagg", bufs=2, space="PSUM"))
        ph = ctx.enter_context(tc.tile_pool(name="ph", bufs=2, space="PSUM"))
        pout = ctx.enter_context(tc.tile_pool(name="pout", bufs=2, space="PSUM"))

        # Resident loads. Two HWDGE rings are used: edge/receiver data on the
        # sync ring, constants + outputs on the scalar ring, so the first
        # edge tiles are not queued behind the big constant transfers.
        # rrel is split head/tail so the first one-hot ops only wait on the
        # small head transfer.
        HEAD = min(PREFETCH, T)
        hc = int(eoff[HEAD]) * 2
        rrel_h = const.tile([P, hc], bf)
        nc.sync.dma_start(rrel_h[:], rrel_d[:, :hc])
        rrel_tl = const.tile([P, EC * 2 - hc], bf)
        iota_t = const.tile([P, Cmax * P], bf)
        nc.scalar.dma_start(iota_t[:], iotac_d[:])
        w1a_t = const.tile([P, HID], bf)
        nc.scalar.dma_start(w1a_t[:], w1a_d[:])
        w1b_t = const.tile([P, HID], bf)
        nc.scalar.dma_start(w1b_t[:], w1b_d[:])
        w2_t = const.tile([P, HB, FO], bf)
        nc.scalar.dma_start(w2_t[:], w2_d[:])
        if with_b1:
            b1r_t = const.tile([1, HID], bf)
            nc.scalar.dma_start(b1r_t[:], b1r_d[:])
        if with_b2:
            b2_t = const.tile([1, FO], bf)
            nc.scalar.dma_start(b2_t[:], b2_d[:])

        def rrel_slice(t):
            a, b = eoff[t] * 2, (eoff[t] + Cs[t]) * 2
            if b <= hc:
                return rrel_h[:, a:b]
            return rrel_tl[:, a - hc : b - hc]
        if with_b1 or with_b2:
            ones_t = const.tile([1, P], bf)
            nc.vector.memset(ones_t[:], 1.0)

        et_tiles = {}

        def load_edges(t):
            c = Cs[t]
            et = epool.tile([P, c * F], bf, tag="et", name=f"et{t}")
            nc.sync.dma_start(et[:], edges_d[:, eoff[t] * F : (eoff[t] + c) * F])
            et_tiles[t] = et

        for t in range(min(PREFETCH, T)):
            load_edges(t)

        nc.sync.dma_start(rrel_tl[:], rrel_d[:, hc:])
        nodesT_t = const.tile([P, NNC], bf)
        nc.scalar.dma_start(nodesT_t[:], nodesT_d[:])

        # Software pipeline across node tiles so the PE never waits on the
        # ScalarE evacuations: iteration t runs scatter(t), layer1(t-1),
        # layer2(t-2) back-to-back on the PE.
        agg_sb = [None] * T
        h_sb = [None] * T

        def stage_scatter(t):
            c = Cs[t]
            et = et_tiles.pop(t)

            # one-hot blocks for the whole node tile in one DVE op; operands
            # viewed as [P, c, 64, 2] so every stream has a step-1 innermost
            # pair (rrel values are duplicated on the host).
            oh = ohpool.tile([P, c, P], bf, tag="oh", name=f"oh{t}")
            rr_bc = (
                rrel_slice(t)
                .rearrange("p (c r) -> p c r", r=2)[:, :, None, :]
                .to_broadcast([P, c, P // 2, 2])
            )
            nc.vector.tensor_tensor(
                out=oh[:].rearrange("p c (q r) -> p c q r", r=2),
                in0=iota_t[:, : c * P].rearrange("p (c q r) -> p c q r", r=2, q=P // 2),
                in1=rr_bc,
                op=mybir.AluOpType.is_equal,
            )

            # scatter-add into agg^T [feat, node] via one-hot matmuls
            agg_ps = pagg.tile([P, P], f32, tag="agg", name=f"agg_ps{t}")
            for j in range(c):
                nc.tensor.matmul(
                    agg_ps[:],
                    lhsT=et[:, j * F : (j + 1) * F],
                    rhs=oh[:, j, :],
                    start=(j == 0),
                    stop=(j == c - 1),
                )
            agg_sb[t] = spool.tile([P, P], bf, tag="agg_sb", name=f"agg_sb{t}")
            nc.scalar.activation(
                agg_sb[t][:], agg_ps[:], mybir.ActivationFunctionType.Copy
            )

        def stage_l1(t):
            h_ps = ph.tile([P, HB, P], f32, tag="h_ps", name=f"h_ps{t}")
            for hb in range(HB):
                nc.tensor.matmul(
                    h_ps[:, hb, :],
                    lhsT=w1a_t[:, hb * P : (hb + 1) * P],
                    rhs=agg_sb[t][:],
                    start=True,
                    stop=False,
                )
                nc.tensor.matmul(
                    h_ps[:, hb, :],
                    lhsT=w1b_t[:, hb * P : (hb + 1) * P],
                    rhs=nodesT_t[:, t * P : (t + 1) * P],
                    start=False,
                    stop=(not with_b1),
                )
                if with_b1:
                    nc.tensor.matmul(
                        h_ps[:, hb, :],
                        lhsT=b1r_t[:, hb * P : (hb + 1) * P],
                        rhs=ones_t[:],
                        start=False,
                        stop=True,
                    )
            h_sb[t] = spool.tile([P, HB, P], bf, tag="h_sb", name=f"h_sb{t}")
            nc.scalar.activation(
                h_sb[t][:], h_ps[:], mybir.ActivationFunctionType.Relu
            )

        def stage_l2(t):
            o_ps = pout.tile([P, FO], f32, tag="o_ps", name=f"o_ps{t}")
            for hb in range(HB):
                nc.tensor.matmul(
                    o_ps[:],
                    lhsT=h_sb[t][:, hb, :],
                    rhs=w2_t[:, hb, :],
                    start=(hb == 0),
                    stop=(hb == HB - 1 and not with_b2),
                )
            if with_b2:
                nc.tensor.matmul(
                    o_ps[:], lhsT=ones_t[:], rhs=b2_t[:], start=False, stop=True
                )
            o_sb = spool.tile([P, FO], f32, tag="o_sb", name=f"o_sb{t}")
            nc.vector.tensor_copy(o_sb[:], o_ps[:])
            nc.sync.dma_start(out_d[t * P : (t + 1) * P, :], o_sb[:])

        for t in range(T + 2):
            if t + PREFETCH < T:
                load_edges(t + PREFETCH)
            if t < T:
                stage_scatter(t)
            if 1 <= t <= T:
                stage_l1(t - 1)
            if 2 <= t <= T + 1:
                stage_l2(t - 2)

    nc.compile()
    return nc


def _get_nc(Cs: tuple, with_b1: bool, with_b2: bool):
    key = (Cs, with_b1, with_b2)
    if key not in _NC_CACHE:
        _NC_CACHE[key] = _build(*key)
    return _NC_CACHE[key]


def kernel(nodes, edge_attr, senders, receivers, W1, b1, W2, b2):
    global LAST_RESULTS
    from concourse.bass_utils import run_bass_kernel_spmd

    bfnp = ml_dtypes.bfloat16
    nodes = np.asarray(nodes, dtype=np.float32)
    edge_attr = np.asarray(edge_attr, dtype=np.float32)
    receivers = np.asarray(receivers, dtype=np.int32)
    W1 = np.asarray(W1, dtype=np.float32)
    b1 = np.asarray(b1, dtype=np.float32)
    W2 = np.asarray(W2, dtype=np.float32)
    b2 = np.asarray(b2, dtype=np.float32)
    with_b1 = bool(np.any(b1 != 0))
    with_b2 = bool(np.any(b2 != 0))

    # ---- host-side prep: sort edges by receiver, order tiles, pad ----
    perm = np.argsort(receivers, kind="stable")
    recv_s = receivers[perm]
    bounds = np.searchsorted(recv_s, np.arange(TILES_PAD + 1) * P)
    counts = np.diff(bounds)                       # [TILES_PAD]
    counts_core = counts.reshape(NCORES, T)
    order = np.argsort(-counts_core, axis=1, kind="stable")  # [NCORES, T]
    sorted_counts = np.take_along_axis(counts_core, order, axis=1)
    Cs = np.maximum(np.ceil(sorted_counts.max(axis=0) / P).astype(int), 1)
    Cs_t = tuple(int(x) for x in Cs)
    Cmax = int(Cs.max())
    eoff = np.concatenate([[0], np.cumsum(Cs)]).astype(int)
    EC = int(eoff[-1])

    # per-tile padded edge data/receiver arrays at width Cmax (then sliced)
    EPT = Cmax * P
    slot = np.arange(EPT)[None, :]
    mask = slot < counts[:, None]  # [TILES_PAD, EPT]
    pidx = np.zeros((TILES_PAD, EPT), np.int64)
    pidx[mask] = perm
    rrel = np.full((TILES_PAD, EPT), -1.0, np.float32)
    rrel[mask] = (recv_s % P).astype(np.float32)

    eb = edge_attr.astype(bfnp)
    g = eb[pidx.reshape(-1)].reshape(TILES_PAD, Cmax, P, F)
    g[~mask.reshape(TILES_PAD, Cmax, P)] = 0
    g = g.reshape(NCORES, T, Cmax, P, F)
    rrel = rrel.reshape(NCORES, T, Cmax, P)

    edges_dev = np.empty((NCORES, P, EC * F), bfnp)
    rr2 = np.empty((NCORES, P, EC * 2), bfnp)
    for c in range(NCORES):
        for t in range(T):
            o = order[c, t]
            w = Cs[t]
            blk = g[c, o, :w]                      # [w, P, F]
            edges_dev[c, :, eoff[t] * F : (eoff[t] + w) * F] = (
                blk.transpose(1, 0, 2).reshape(P, w * F)
            )
            rb = rrel[c, o, :w].astype(bfnp)       # [w, P]
            rr2[c, :, eoff[t] * 2 : (eoff[t] + w) * 2] = np.repeat(
                rb.T, 2, axis=1
            )

    iotac = np.tile(np.arange(P, dtype=np.float32), Cmax)[None].repeat(P, 0)
    iotac = iotac.astype(bfnp)                     # [P, Cmax*P]

    nodes_pad = np.zeros((NPAD, F), np.float32)
    nodes_pad[:N_NODES] = nodes
    nodes_core = nodes_pad.reshape(NCORES, T, P, F)
    nodesT_dev = np.empty((NCORES, P, NNC), bfnp)
    for c in range(NCORES):
        sel = nodes_core[c, order[c]].reshape(NNC, F)  # position-ordered
        nodesT_dev[c] = sel.T.astype(bfnp)

    w1a = np.ascontiguousarray(W1[:F]).astype(bfnp)
    w1b = np.ascontiguousarray(W1[F:]).astype(bfnp)
    w2 = np.ascontiguousarray(W2.reshape(HB, P, FO).transpose(1, 0, 2)).astype(bfnp)

    nc = _get_nc(Cs_t, with_b1, with_b2)

    in_maps = []
    for c in range(NCORES):
        m = {
            "edges": edges_dev[c],
            "rrel": rr2[c],
            "iotac": iotac,
            "nodesT": nodesT_dev[c],
            "w1a": w1a,
            "w1b": w1b,
            "w2": w2,
        }
        if with_b1:
            m["b1r"] = b1.reshape(1, HID).astype(bfnp)
        if with_b2:
            m["b2"] = b2.reshape(1, FO).astype(bfnp)
        in_maps.append(m)

    trace = bool(int(os.environ.get("GNN_TRACE", "0")))
    res = run_bass_kernel_spmd(
        nc,
        in_maps,
        core_ids=list(range(NCORES)),
        trace=trace,
    )
    LAST_RESULTS = res

    # un-permute tile positions back to original node order
    out = np.empty((NPAD, FO), np.float32)
    for c in range(NCORES):
        r = res.results[c]["out"].reshape(T, P, FO)
        out.reshape(NCORES, T, P, FO)[c, order[c]] = r
    return np.ascontiguousarray(out[:N_NODES])
